# revision 17
# baseline (speedup 1.0000x reference)
"""Block-diagonal masked cross-attention kernel for Trainium2 (8 NeuronCores).

Strategy: rows of x_mole are sharded 1024/core. The batch mask is
block-diagonal (batch is sorted), so row r only attends to a contiguous
block of columns around r. Each core receives a host-gathered window of
x_conf rows [1024*i - SHIFT, +WINDOW) and each 128-row tile t computes its
attention over a fixed 512-wide sub-window at window offset 128*t, which
covers every row's block as long as no block exceeds the slack (checked on
host; wider fallback geometries compile on demand). The full N x N attn
output is zero outside those bands, so the host scatters the per-tile
[128, WIDTH] bands into a zero array.

All matmuls run as float32r (full PE rate at N>=256). Softmax is computed
shift-free: exp(dot) * mask / sum(exp(dot) * mask), which is mathematically
identical to the reference's max-shifted version (softmax shift invariance;
|dot| is bounded ~35 here so no overflow).
"""

import numpy as np

N, D, NCORES = 8192, 512, 8
RPC = N // NCORES  # rows per core
NT = RPC // 128    # 128-row tiles per core
EPS = 1e-5

_CACHE = {}


def _segments(total, chunk=512):
    segs = []
    off = 0
    while off < total:
        s = min(chunk, total - off)
        segs.append((off, s))
        off += s
    return segs


def _build(width, use_ln_affine):
    from contextlib import ExitStack

    import concourse.bass as bass
    import concourse.tile as tile
    from concourse import bacc, mybir
    from concourse.masks import make_identity

    f32 = mybir.dt.float32
    f32r = mybir.dt.float32r
    AFT = mybir.ActivationFunctionType
    ALU = mybir.AluOpType

    window = 128 * (NT - 1) + width  # last tile's sub-window must fit
    nw = window // 128

    nc = bacc.Bacc(
        "TRN2",
        target_bir_lowering=False,
        debug=False,
        enable_asserts=False,
        num_devices=NCORES,
    )

    xm_d = nc.dram_tensor("xm", [RPC, D], f32, kind="ExternalInput")
    xcw_d = nc.dram_tensor("xcw", [window, D], f32r, kind="ExternalInput")
    cbw_d = nc.dram_tensor("cbw", [window], f32, kind="ExternalInput")
    rb_d = nc.dram_tensor("rb", [RPC], f32, kind="ExternalInput")
    w1t_d = nc.dram_tensor("w1t", [D, D], f32r, kind="ExternalInput")
    w2t_d = nc.dram_tensor("w2t", [D, D], f32r, kind="ExternalInput")
    r1t_d = nc.dram_tensor("r1t", [D, D], f32r, kind="ExternalInput")
    r2n_d = nc.dram_tensor("r2n", [D, D], f32r, kind="ExternalInput")
    u1_d = nc.dram_tensor("u1", [D], f32, kind="ExternalInput")
    u2_d = nc.dram_tensor("u2", [D], f32, kind="ExternalInput")
    b1_d = nc.dram_tensor("b1", [D], f32, kind="ExternalInput")
    b2_d = nc.dram_tensor("b2", [D], f32r, kind="ExternalInput")
    if use_ln_affine:
        lnw_d = nc.dram_tensor("lnw", [D], f32, kind="ExternalInput")
        lnb_d = nc.dram_tensor("lnb", [D], f32, kind="ExternalInput")
    attn_d = nc.dram_tensor("attn_o", [RPC, width], f32, kind="ExternalOutput")
    xe_d = nc.dram_tensor("xe_o", [RPC, D], f32, kind="ExternalOutput")

    xm_t = xm_d.ap().rearrange("(t p) d -> t p d", p=128)
    xcw_t = xcw_d.ap().rearrange("(w p) d -> w p d", p=128)
    attn_t = attn_d.ap().rearrange("(t p) w -> t p w", p=128)
    xe_t = xe_d.ap().rearrange("(t p) d -> t p d", p=128)

    nck = width // 128  # em-transpose chunks per row tile
    mm_bufs = 4 if width <= 512 else 2
    tr_bufs = 4 if width <= 512 else 2

    with tile.TileContext(nc) as tc, ExitStack() as ctx:
        const = ctx.enter_context(tc.tile_pool(name="const", bufs=1))
        big = ctx.enter_context(tc.tile_pool(name="big", bufs=1))
        p_xcT = ctx.enter_context(tc.tile_pool(name="xcT", bufs=2))
        p_xln = ctx.enter_context(tc.tile_pool(name="xln", bufs=3))
        p_st = ctx.enter_context(tc.tile_pool(name="st", bufs=6))
        p_msk = ctx.enter_context(tc.tile_pool(name="msk", bufs=2))
        p_em = ctx.enter_context(tc.tile_pool(name="em", bufs=2))
        p_emT = ctx.enter_context(tc.tile_pool(name="emT", bufs=3))
        p_attn = ctx.enter_context(tc.tile_pool(name="attn", bufs=2))
        p_xw = ctx.enter_context(tc.tile_pool(name="xw", bufs=3))
        p_xwT = ctx.enter_context(tc.tile_pool(name="xwT", bufs=2))
        p_h1T = ctx.enter_context(tc.tile_pool(name="h1T", bufs=2))
        p_h2 = ctx.enter_context(tc.tile_pool(name="h2", bufs=2))
        p_xe = ctx.enter_context(tc.tile_pool(name="xe", bufs=2))
        p_mm = ctx.enter_context(tc.tile_pool(name="mm", bufs=mm_bufs, space="PSUM"))
        p_tr = ctx.enter_context(tc.tile_pool(name="tr", bufs=tr_bufs, space="PSUM"))

        ident = const.tile([128, 128], f32)
        make_identity(nc, ident)
        epsb = const.tile([128, 1], f32)
        nc.vector.memset(epsb, EPS)
        ones1r_f = const.tile([1, 128], f32)
        nc.vector.memset(ones1r_f, 1.0)
        ones1r = const.tile([1, 128], f32r)
        nc.vector.tensor_copy(out=ones1r, in_=ones1r_f)

        cbb = const.tile([128, window], f32)
        cb_ap = cbw_d.ap()
        nc.gpsimd.dma_start(
            out=cbb,
            in_=bass.AP(tensor=cb_ap.tensor, offset=cb_ap.offset, ap=[[0, 128]] + cb_ap.ap),
        )
        rbs = const.tile([128, NT], f32)
        nc.sync.dma_start(out=rbs, in_=rb_d.ap().rearrange("(t p) -> p t", p=128))

        w1t = const.tile([128, 4, D], f32r)
        nc.sync.dma_start(out=w1t, in_=w1t_d.ap().rearrange("(k p) j -> p k j", p=128))
        w2t = const.tile([128, 4, D], f32r)
        nc.sync.dma_start(out=w2t, in_=w2t_d.ap().rearrange("(k p) j -> p k j", p=128))
        r1t = const.tile([128, 4, D], f32r)
        nc.sync.dma_start(out=r1t, in_=r1t_d.ap().rearrange("(k p) j -> p k j", p=128))
        r2n = const.tile([128, 4, D], f32r)
        nc.sync.dma_start(out=r2n, in_=r2n_d.ap().rearrange("(k p) j -> p k j", p=128))
        u1s = const.tile([128, 4], f32)
        nc.sync.dma_start(out=u1s, in_=u1_d.ap().rearrange("(k p) -> p k", p=128))
        u2s = const.tile([128, 4], f32)
        nc.sync.dma_start(out=u2s, in_=u2_d.ap().rearrange("(k p) -> p k", p=128))
        b1s = const.tile([128, 4], f32)
        nc.sync.dma_start(out=b1s, in_=b1_d.ap().rearrange("(k p) -> p k", p=128))
        b2r = const.tile([1, D], f32r)
        nc.sync.dma_start(out=b2r, in_=b2_d.ap()[None, :])
        if use_ln_affine:
            lnwb = const.tile([128, D], f32)
            lw_ap = lnw_d.ap()
            nc.gpsimd.dma_start(
                out=lnwb,
                in_=bass.AP(tensor=lw_ap.tensor, offset=lw_ap.offset, ap=[[0, 128]] + lw_ap.ap),
            )
            lnbb = const.tile([128, D], f32)
            lb_ap = lnb_d.ap()
            nc.gpsimd.dma_start(
                out=lnbb,
                in_=bass.AP(tensor=lb_ap.tensor, offset=lb_ap.offset, ap=[[0, 128]] + lb_ap.ap),
            )

        xcn = big.tile([128, nw, D], f32r)   # window rows, normal layout
        kT = big.tile([128, 4, window], f32r)
        qT = big.tile([128, 4, RPC], f32r)

        def ln_stats(src):
            """Return (mu, rs) access patterns for a [128, 512] tile."""
            stats = p_st.tile([128, 6], f32, tag="stats")
            nc.vector.bn_stats(out=stats, in_=src)
            mv = p_st.tile([128, 2], f32, tag="mv")
            nc.vector.bn_aggr(out=mv, in_=stats)
            sd = p_st.tile([128, 1], f32, tag="sd")
            nc.scalar.activation(out=sd, in_=mv[:, 1:2], func=AFT.Sqrt, bias=epsb[:, 0:1], scale=1.0)
            rs = p_st.tile([128, 1], f32, tag="rs")
            nc.vector.reciprocal(out=rs, in_=sd)
            return mv[:, 0:1], rs[:, 0:1]

        def proj_block(dst, dst_off, seg_n, src_tiles, wts, us):
            """dst[:, jj, dst_off:+seg_n] += wts.T @ LN(src rows) for 4 j-chunks.

            src_tiles: list of (normal_tile_ap, ncols) [128, 512] raw inputs
            whose LN'd transposes form the rhs [K=d, N=seg_n].
            """
            xT = p_xcT.tile([128, 4, 512], f32r, tag="xT")
            col = 0
            for src in src_tiles:
                mu, rs = ln_stats(src)
                xln = p_xln.tile([128, D], f32, tag="xln")
                nc.vector.tensor_scalar(
                    out=xln, in0=src, scalar1=mu, scalar2=rs,
                    op0=ALU.subtract, op1=ALU.mult,
                )
                for dk in range(4):
                    pt = p_tr.tile([128, 128], f32, tag="tr")
                    nc.tensor.transpose(pt, xln[:, 128 * dk:128 * dk + 128], ident)
                    nc.vector.tensor_copy(out=xT[:, dk, col:col + 128], in_=pt)
                col += 128
            for jj in range(4):
                ps = p_mm.tile([128, width], f32, tag="mm")
                for dk in range(4):
                    nc.tensor.matmul(
                        ps[:, 0:seg_n],
                        wts[:, dk, 128 * jj:128 * jj + 128],
                        xT[:, dk, 0:seg_n],
                        start=(dk == 0), stop=(dk == 3),
                    )
                nc.scalar.activation(
                    out=dst[:, jj, dst_off:dst_off + seg_n], in_=ps[:, 0:seg_n],
                    func=AFT.Identity, bias=us[:, jj:jj + 1], scale=1.0,
                )

        # ---- k path over the window ----
        for (off, seg_n) in _segments(window):
            tiles = []
            for q in range(seg_n // 128):
                w = off // 128 + q
                nc.sync.dma_start(out=xcn[:, w, :], in_=xcw_t[w])
                tiles.append(xcn[:, w, :].bitcast(f32))
            proj_block(kT, off, seg_n, tiles, w2t, u2s)

        # ---- q path over own rows ----
        for (off, seg_n) in _segments(RPC):
            tiles = []
            for q in range(seg_n // 128):
                t = off // 128 + q
                xmn = p_xln.tile([128, D], f32, tag="xmn")
                nc.sync.dma_start(out=xmn, in_=xm_t[t])
                tiles.append(xmn)
            proj_block(qT, off, seg_n, tiles, w1t, u1s)

        # ---- attention + MLP, per 512-row block ----
        for B in range(NT // 4):
            xwT = p_xwT.tile([128, 4, 512], f32r, tag="xwT")
            for q4 in range(4):
                t = 4 * B + q4
                ps_dot = p_mm.tile([128, width], f32, tag="mm")
                for (s0, sn) in _segments(width):
                    for jj in range(4):
                        nc.tensor.matmul(
                            ps_dot[:, s0:s0 + sn],
                            qT[:, jj, 128 * t:128 * t + 128],
                            kT[:, jj, 128 * t + s0:128 * t + s0 + sn],
                            start=(jj == 0), stop=(jj == 3),
                        )
                mask = p_msk.tile([128, width], f32, tag="mask")
                nc.vector.tensor_scalar(
                    out=mask, in0=cbb[:, 128 * t:128 * t + width],
                    scalar1=rbs[:, t:t + 1], scalar2=None, op0=ALU.is_equal,
                )
                e = p_attn.tile([128, width], f32, tag="e")
                nc.scalar.activation(out=e, in_=ps_dot, func=AFT.Exp)
                em = p_em.tile([128, width], f32, tag="em")
                nc.vector.tensor_tensor(out=em, in0=e, in1=mask, op=ALU.mult)

                ps_xw = p_mm.tile([128, width], f32, tag="mm")
                for ck in range(nck):
                    pt = p_tr.tile([128, 128], f32, tag="tr")
                    nc.tensor.transpose(pt, em[:, 128 * ck:128 * ck + 128], ident)
                    emT = p_emT.tile([128, 128], f32r, tag="emT")
                    nc.vector.tensor_copy(out=emT, in_=pt)
                    nc.tensor.matmul(
                        ps_xw[:, 0:D], emT, xcn[:, t + ck, :],
                        start=(ck == 0), stop=(ck == nck - 1),
                    )
                den = p_st.tile([128, 1], f32, tag="den")
                nc.vector.tensor_reduce(
                    out=den, in_=em, axis=mybir.AxisListType.X, op=ALU.add,
                )
                rcp = p_st.tile([128, 1], f32, tag="rcp")
                nc.vector.reciprocal(out=rcp, in_=den)
                attn_sb = p_attn.tile([128, width], f32, tag="attn")
                nc.scalar.activation(out=attn_sb, in_=em, func=AFT.Copy, scale=rcp[:, 0:1])
                nc.sync.dma_start(out=attn_t[t], in_=attn_sb)
                xw = p_xw.tile([128, D], f32, tag="xw")
                nc.scalar.activation(out=xw, in_=ps_xw[:, 0:D], func=AFT.Copy, scale=rcp[:, 0:1])
                for dk in range(4):
                    pt = p_tr.tile([128, 128], f32, tag="tr")
                    nc.tensor.transpose(pt, xw[:, 128 * dk:128 * dk + 128], ident)
                    nc.vector.tensor_copy(out=xwT[:, dk, 128 * q4:128 * q4 + 128], in_=pt)


            h1T = p_h1T.tile([128, 4, 512], f32r, tag="h1T")
            for jj in range(4):
                ps = p_mm.tile([128, width], f32, tag="mm")
                for dk in range(4):
                    nc.tensor.matmul(
                        ps[:, 0:512],
                        r1t[:, dk, 128 * jj:128 * jj + 128],
                        xwT[:, dk, :],
                        start=(dk == 0), stop=(dk == 3),
                    )
                nc.scalar.activation(
                    out=h1T[:, jj, :], in_=ps[:, 0:512],
                    func=AFT.Silu, bias=b1s[:, jj:jj + 1], scale=1.0,
                )
            for q4 in range(4):
                t = 4 * B + q4
                ps = p_mm.tile([128, width], f32, tag="mm")
                for jj in range(4):
                    nc.tensor.matmul(
                        ps[:, 0:D],
                        h1T[:, jj, 128 * q4:128 * q4 + 128],
                        r2n[:, jj, :],
                        start=(jj == 0), stop=False,
                    )
                nc.tensor.matmul(
                    ps[:, 0:D], ones1r, b2r,
                    start=False, stop=True,
                )
                h2 = p_h2.tile([128, D], f32, tag="h2")
                nc.scalar.activation(out=h2, in_=ps[:, 0:D], func=AFT.Silu)
                mu2, rs2 = ln_stats(h2)
                xe_sb = p_xe.tile([128, D], f32, tag="xe")
                nc.vector.tensor_scalar(
                    out=xe_sb, in0=h2, scalar1=mu2, scalar2=rs2,
                    op0=ALU.subtract, op1=ALU.mult,
                )
                if use_ln_affine:
                    nc.vector.tensor_tensor(out=xe_sb, in0=xe_sb, in1=lnwb, op=ALU.mult)
                    nc.vector.tensor_tensor(out=xe_sb, in0=xe_sb, in1=lnbb, op=ALU.add)
                nc.sync.dma_start(out=xe_t[t], in_=xe_sb)

    nc.compile()
    return nc


def _choose_width(batch):
    starts = np.searchsorted(batch, batch, side="left")
    ends = np.searchsorted(batch, batch, side="right")
    r = np.arange(N)
    t = (r % RPC) // 128
    core = r // RPC
    for width in (512, 768, 1024, 1280, 1536, 2048, 4096):
        shift = ((width - 128) // 2)
        w0 = core * RPC - shift + 128 * t
        if ((starts >= w0) & (ends <= w0 + width)).all():
            return width, shift
    raise ValueError("no geometry covers the batch blocks")


def kernel(x_mole, x_conf, W1, W2, phi1_w, phi1_b, phi2_w, phi2_b,
           rho_w1, rho_b1, rho_w2, rho_b2, rho_ln_w, rho_ln_b, batch):
    from concourse.bass_utils import run_bass_kernel_spmd

    f32 = np.float32
    x_mole = np.ascontiguousarray(np.asarray(x_mole, dtype=f32))
    x_conf = np.ascontiguousarray(np.asarray(x_conf, dtype=f32))
    W1 = np.asarray(W1, dtype=f32)
    W2 = np.asarray(W2, dtype=f32)
    phi1_w = np.asarray(phi1_w, dtype=f32)
    phi1_b = np.asarray(phi1_b, dtype=f32)
    phi2_w = np.asarray(phi2_w, dtype=f32)
    phi2_b = np.asarray(phi2_b, dtype=f32)
    rho_w1 = np.asarray(rho_w1, dtype=f32)
    rho_b1 = np.asarray(rho_b1, dtype=f32)
    rho_w2 = np.asarray(rho_w2, dtype=f32)
    rho_b2 = np.asarray(rho_b2, dtype=f32)
    rho_ln_w = np.asarray(rho_ln_w, dtype=f32)
    rho_ln_b = np.asarray(rho_ln_b, dtype=f32)
    batch = np.asarray(batch)

    width, shift = _choose_width(batch)
    window = 128 * (NT - 1) + width
    use_ln_affine = not (np.all(rho_ln_w == 1.0) and np.all(rho_ln_b == 0.0))

    key = (width, use_ln_affine)
    if key not in _CACHE:
        _CACHE[key] = _build(width, use_ln_affine)
    nc = _CACHE[key]

    # host-folded weights (torch Linear: y = x @ W.T)
    w1t = np.ascontiguousarray((W1 * phi1_w[None, :]).T)
    w2t = np.ascontiguousarray((W2 * phi2_w[None, :]).T)
    u1 = (phi1_b @ W1.T).astype(f32)
    u2 = (phi2_b @ W2.T).astype(f32)
    r1t = np.ascontiguousarray(rho_w1.T)
    r2n = np.ascontiguousarray(rho_w2.T)

    batch_f = batch.astype(f32)
    in_maps = []
    for i in range(NCORES):
        w0 = i * RPC - shift
        lo, hi = max(0, w0), min(N, w0 + window)
        xcw = np.zeros((window, D), dtype=f32)
        xcw[lo - w0:hi - w0] = x_conf[lo:hi]
        cbw = np.full(window, -1.0, dtype=f32)
        cbw[lo - w0:hi - w0] = batch_f[lo:hi]
        m = {
            "xm": x_mole[i * RPC:(i + 1) * RPC],
            "xcw": xcw,
            "cbw": cbw,
            "rb": np.ascontiguousarray(batch_f[i * RPC:(i + 1) * RPC]),
            "w1t": w1t, "w2t": w2t, "r1t": r1t, "r2n": r2n,
            "u1": u1, "u2": u2, "b1": rho_b1, "b2": rho_b2,
        }
        if use_ln_affine:
            m["lnw"] = rho_ln_w
            m["lnb"] = rho_ln_b
        in_maps.append(m)

    res = run_bass_kernel_spmd(nc, in_maps, core_ids=list(range(NCORES)))

    xe = np.concatenate([res.results[i]["xe_o"] for i in range(NCORES)], axis=0)
    attn = np.zeros((N, N), dtype=f32)
    for i in range(NCORES):
        a = res.results[i]["attn_o"]
        for t in range(NT):
            r0 = i * RPC + 128 * t
            c0 = i * RPC - shift + 128 * t
            lo, hi = max(0, c0), min(N, c0 + width)
            if lo < hi:
                attn[r0:r0 + 128, lo:hi] = a[128 * t:128 * t + 128, lo - c0:hi - c0]
    return xe, attn
